# revision 11
# baseline (speedup 1.0000x reference)
"""Trainium2 Bass kernel for nn_BatchedCauchyKernel (fp8 DoubleRow, v4).

Computes, for x[N,D], y[M,D], sample_x[N,S], sample_y[M,S], scale[S]:
    d[i,j]   = |x_i|^2 + |y_j|^2 - 2 x_i.y_j
    sx_i     = clip(softplus(sample_x_i . scale), 1e-10, 1e4)
    sy_j     = clip(softplus(sample_y_j . scale), 1e-10, 1e4)
    res      = 1 / (1 + d / sqrt(sx_i * sy_j))
    out      = res * sigmoid(phi * (res - clip(cutoff, 0, 1000)))

Sharding: 2D grid over 8 cores, 4 x-blocks (NS=2048) x 2 y-blocks (MS=2048).

Core algebra: with rsx = sx^-1/2, rsy = sy^-1/2 and the mask sigmoid
linearized as m0 + m1*res (baseline trick), let
    P_ij = rsy_j*|x_i|^2 + rsy_j*|y_j|^2*1 - 2*rsy_j*(x_i.y_j)
computed as ONE fp8 DoubleRow matmul chain: xp8 = fp8(-2*x) (constant-scale
ACT cast), yp8 = fp8(rsy*y), plus a K=3 fp16 extension matmul
    lhsT rows [sqx_hi, sqx_lo, 1] x rhs rows [rsy16, rsy16, b16=rsy*sqy].
Then the ACT reciprocal does the rest with its per-partition scale/bias:
    r = Recip(P * (inv_m1*rsx_i) + inv_m1) = sqrt(m1)*res
    out = (r + c0) * r = m0*res + m1*res^2            (DVE stt, fp16)
rsx enters EXACTLY (f32 recip scale) - no x-side quant scaling at all.

numpy-sim max rel err of this scheme: 9.9e-3 (gate 2e-2).

Perf learnings baked in (v2 = 145us, v3 = 244us):
  * PE DVFS needs >3us continuous bursts to ramp: keep full-po [128,2048]
    psum groups (12-matmul bursts), ONE recip + ONE stt per po.
  * All DRAM roundtrips must be contiguous or >=32B-run descriptor
    patterns ([128,W]<->"(pi po)" views, [1,W] rows). The v3 packed
    [7,NS] "v-innermost" write generated 2-byte descriptors and stalled
    the whole device for 70us.
  * Partition broadcasts as 0-stride DMA reads (HW-verified in v3).
  * x-side loads/roundtrips + output stores on sync ring; y-side on the
    scalar ring; ACT does softplus + x-casts + x-squares + psum row
    copies + recips; DVE does y-squares + y-quant + tiny vec ops + stt.
"""

import os
import sys

sys.path.insert(0, "/opt/trn_rl_repo")

import numpy as np
import ml_dtypes

N, M, D, S = 8192, 4096, 512, 16
XB, YB = 4, 2  # core grid
CORES = XB * YB
NS = N // XB  # 2048 rows of x per core
MS = M // YB  # 2048 rows of y per core
PO = NS // 128  # 16 i-tiles
BW = MS // 128  # 16
JT = MS // 512  # 4 j-tiles of 512
KT = D // 128  # 4 k-tiles
KQ = KT // 2  # 2 DoubleRow k-pair groups

SOFTPLUS_MIN = 1e-10
SOFTPLUS_MAX = 10000.0

_CACHE = {}


def _act_recip(nc, out, in_, scale, bias):
    """r = Reciprocal(in_*scale + bias); scale may be a [128,1] AP."""
    import concourse.mybir as mybir

    eng = nc.scalar
    inputs = [eng.lower_ap(in_)]
    inputs.append(mybir.ImmediateValue(dtype=mybir.dt.float32, value=bias))
    if isinstance(scale, float):
        inputs.append(mybir.ImmediateValue(dtype=mybir.dt.float32, value=scale))
    else:
        inputs.append(eng.lower_ap(scale))
    inputs.append(mybir.ImmediateValue(dtype=mybir.dt.float32, value=0.0))
    return eng.add_instruction(
        mybir.InstActivation(
            name=nc.get_next_instruction_name(),
            func=mybir.ActivationFunctionType.Reciprocal,
            ins=inputs,
            outs=[eng.lower_ap(out)],
        )
    )


def _fit_mask_linear(phi_val, cutoff_val, R=0.15):
    # res*sigmoid(phi*(res-c)) ~= m0*res + m1*res^2 for res in [0,R]
    t = (np.cos(np.linspace(0, np.pi, 2001)) + 1) * (R / 2)
    g = 1.0 / (1.0 + np.exp(-phi_val * (t - cutoff_val)))
    m1_, m0_ = np.polyfit(t, g, 1)
    gerr = np.abs(np.polyval([m1_, m0_], t) - g) / np.abs(g)
    assert gerr.max() < 2e-3, f"mask linearization too coarse: {gerr.max()}"
    return float(m0_), float(m1_)


def _build(phi_val: float, cutoff_val: float, R: float = 0.15):
    import concourse.mybir as mybir
    import concourse.tile as tile
    from concourse import bacc

    dt = mybir.dt
    AF = mybir.ActivationFunctionType
    OP = mybir.AluOpType
    DR = mybir.MatmulPerfMode.DoubleRow

    m0, m1 = _fit_mask_linear(phi_val, cutoff_val, R)
    inv_m1 = 1.0 / float(np.sqrt(m1))
    c0 = m0 / float(np.sqrt(m1))

    nc = bacc.Bacc("TRN2", target_bir_lowering=False)

    xT_d = nc.dram_tensor("xT_shard", [D, NS], dt.bfloat16, kind="ExternalInput")
    yT_d = nc.dram_tensor("yT_shard", [D, MS], dt.bfloat16, kind="ExternalInput")
    sx_d = nc.dram_tensor("sample_x_shard", [NS, S], dt.float32, kind="ExternalInput")
    sy_d = nc.dram_tensor("sample_y_shard", [MS, S], dt.float32, kind="ExternalInput")
    sc_d = nc.dram_tensor("scale_full", [1, S], dt.float32, kind="ExternalInput")
    out_d = nc.dram_tensor("out_shard", [NS, MS], dt.float16, kind="ExternalOutput")

    xT_v = xT_d.rearrange("(kt kp) i -> kp kt i", kp=128)  # [128, KT, NS]
    yT_v = yT_d.rearrange("(kt kp) j -> kp kt j", kp=128)  # [128, KT, MS]
    # sample_x PO-MAJOR: rsx[p, po] <-> row i = po*128 + p, so the recip
    # scale AP for psum block po is rsx_s[:, po:po+1].
    sx_v = sx_d.rearrange("(po pi) s -> pi po s", pi=128)  # [128, PO, S]
    sy_v = sy_d.rearrange("(a b) s -> a b s", a=128)  # [128, BW, S], j = a*BW+b
    out_v = out_d.rearrange("(po pi) j -> pi po j", pi=128)  # [128, PO, MS]

    with tile.TileContext(nc) as tc:
        with (
            tc.tile_pool(name="persist", bufs=1) as persist,
            tc.tile_pool(name="dram", bufs=1, space="DRAM") as dram,
            tc.tile_pool(name="psum", bufs=2, space="PSUM") as psum_p,
            tc.tile_pool(name="sqscr", bufs=8) as sqscr,
            tc.tile_pool(name="main", bufs=3) as main,
        ):
            # ---------------- loads ----------------
            ssx = persist.tile([128, PO, S], dt.float32)
            nc.sync.dma_start(ssx[:], sx_v)
            ssy = persist.tile([128, BW, S], dt.float32)
            nc.sync.dma_start(ssy[:], sy_v)
            scale_rep = persist.tile([128, S], dt.float32)
            nc.scalar.dma_start(scale_rep[:], sc_d[0:1, :].to_broadcast((128, S)))

            xT_sb = persist.tile([128, KT, NS], dt.bfloat16)
            yT_sb = persist.tile([128, KT, MS], dt.bfloat16)
            for kt in range(KT):
                nc.sync.dma_start(xT_sb[:, kt, :], xT_v[:, kt, :])
                nc.scalar.dma_start(yT_sb[:, kt, :], yT_v[:, kt, :])

            # xp8 = fp8(x) straight cast during the DMA itself (SWDGE);
            # the -2 dot factor is folded into the y-side quant scale.
            xp8 = persist.tile([128, KT, NS], dt.float8e4)
            nc.gpsimd.dma_start(xp8[:], xT_v)

            # ---------------- softplus chains ----------------
            def softplus_rsqrt(ss, width, tag):
                tmp = persist.tile([128, width, S], dt.float32, tag=f"tmp_{tag}")
                nc.vector.tensor_tensor(
                    tmp[:], ss[:],
                    scale_rep[:, None, :].to_broadcast((128, width, S)), OP.mult,
                )
                red = persist.tile([128, width], dt.float32, tag=f"red_{tag}")
                nc.vector.tensor_reduce(
                    red[:, :, None], tmp[:], mybir.AxisListType.X, OP.add
                )
                v = persist.tile([128, width], dt.float32, tag=f"v_{tag}")
                nc.scalar.activation(v[:], red[:], AF.Exp)
                nc.scalar.activation(v[:], v[:], AF.Ln, bias=1.0)
                nc.vector.tensor_scalar(
                    v[:], v[:], SOFTPLUS_MAX, SOFTPLUS_MIN, OP.min, OP.max
                )
                rs = persist.tile([128, width], dt.float32, tag=f"rs_{tag}")
                nc.scalar.activation(rs[:], v[:], AF.Ln)
                nc.scalar.activation(rs[:], rs[:], AF.Exp, scale=-0.5)
                return rs

            rsy = softplus_rsqrt(ssy, BW, "y")  # [128, BW] a-major (first!)
            rsx = softplus_rsqrt(ssx, PO, "x")  # [128, PO] po-major

            # y quant scale row (-2 folded in) -> 0-stride broadcast
            ry_bf = persist.tile([128, BW], dt.bfloat16)
            nc.vector.tensor_scalar_mul(ry_bf[:], rsy[:], -2.0)
            d_ry = dram.tile([1, MS], dt.bfloat16)
            nc.scalar.dma_start(d_ry[0, :].rearrange("(a b) -> a b", a=128), ry_bf[:])
            ry_bc = persist.tile([128, MS], dt.bfloat16)
            nc.scalar.dma_start(ry_bc[:], d_ry[0:1, :].to_broadcast((128, MS)))

            # recip scale = inv_m1 * rsx (consumed as [128,1] slices)
            rsx_s = persist.tile([128, PO], dt.float32)
            nc.vector.tensor_scalar_mul(rsx_s[:], rsx[:], inv_m1)

            # ---------------- squares / quant (kt-pipelined) --------
            ones_bf = persist.tile([128, 128], dt.bfloat16)
            nc.vector.memset(ones_bf[:], 1.0)
            yp8 = persist.tile([128, KT, MS], dt.float8e4)
            sqy_ps = psum_p.tile([128, 2048], dt.float32, tag="acc", name="sqy_ps")
            sqx_ps = psum_p.tile([128, 2048], dt.float32, tag="acc", name="sqx_ps")

            # ACT: x squares only
            sq_xs, sq_ys = [], []
            for kt in range(KT):
                sq_x = sqscr.tile([128, NS], dt.bfloat16, tag="sq")
                nc.scalar.activation(sq_x[:], xT_sb[:, kt, :], AF.Square)
                sq_xs.append(sq_x)

            # DVE: y squares first (only need yT), then quant behind ry_bc
            for kt in range(KT):
                sq_y = sqscr.tile([128, MS], dt.bfloat16, tag="sq")
                nc.vector.tensor_tensor(
                    sq_y[:], yT_sb[:, kt, :], yT_sb[:, kt, :], OP.mult
                )
                sq_ys.append(sq_y)
            for kt in range(3):
                nc.vector.tensor_tensor(
                    yp8[:, kt, :], yT_sb[:, kt, :], ry_bc[:], OP.mult
                )

            # PE: ones-contractions (y first - DVE squares land earlier)
            for kt in range(KT):
                for c in range(4):
                    nc.tensor.matmul(
                        sqy_ps[:, c * 512:(c + 1) * 512],
                        lhsT=ones_bf[:],
                        rhs=sq_ys[kt][:, c * 512:(c + 1) * 512],
                        start=(kt == 0),
                        stop=(kt == KT - 1),
                    )
            for kt in range(KT):
                for c in range(4):
                    nc.tensor.matmul(
                        sqx_ps[:, c * 512:(c + 1) * 512],
                        lhsT=ones_bf[:],
                        rhs=sq_xs[kt][:, c * 512:(c + 1) * 512],
                        start=(kt == 0),
                        stop=(kt == KT - 1),
                    )

            # psum rows -> DRAM -> [128, W] small layouts
            sq_row_y = persist.tile([1, MS], dt.float32)
            nc.scalar.activation(sq_row_y[:, 0:1024], sqy_ps[0:1, 0:1024], AF.Copy)
            nc.scalar.activation(sq_row_y[:, 1024:2048], sqy_ps[0:1, 1024:2048],
                                 AF.Copy)
            sq_row_x = persist.tile([1, NS], dt.float32)
            nc.scalar.activation(sq_row_x[:, 0:1024], sqx_ps[0:1, 0:1024], AF.Copy)
            nc.scalar.activation(sq_row_x[:, 1024:2048], sqx_ps[0:1, 1024:2048],
                                 AF.Copy)
            d_sqy = dram.tile([1, MS], dt.float32)
            nc.scalar.dma_start(d_sqy[:, :], sq_row_y[:])
            sqy_pp = persist.tile([128, BW], dt.float32)
            nc.scalar.dma_start(
                sqy_pp[:], d_sqy[0, :].rearrange("(a b) -> a b", a=128)
            )
            d_sqx = dram.tile([1, NS], dt.float32)
            nc.sync.dma_start(d_sqx[:, :], sq_row_x[:])
            sqx_pp = persist.tile([128, PO], dt.float32)
            nc.sync.dma_start(
                sqx_pp[:], d_sqx[0, :].rearrange("(pi po) -> pi po", pi=128)
            )

            # ---------------- ext vectors (fp8 multi-level) ----------------
            # lhsT rows: [sq0, sq0, sq1, sq1, sq2, 4, 4, (4)] of sqx/8 levels
            # rhs  rows: [ry0, ry1, ry0, ry1, ry0, b0, b1, 0] of 8*rsy / b/4
            # so ext = sqx*rsy + rsy*sqy exactly to fp8 level-3/2 residuals.
            def levels(src_pool, vec, width, n, tag):
                out = []
                rem = vec
                for li in range(n):
                    h8 = persist.tile([128, width], dt.float8e4, tag=f"{tag}h{li}")
                    nc.vector.tensor_copy(h8[:], rem[:])
                    out.append(h8)
                    if li < n - 1:
                        hf = persist.tile([128, width], dt.float32,
                                          tag=f"{tag}f{li}")
                        nc.vector.tensor_copy(hf[:], h8[:])
                        r2 = persist.tile([128, width], dt.float32,
                                          tag=f"{tag}r{li}")
                        nc.vector.tensor_tensor(r2[:], rem[:], hf[:], OP.subtract)
                        rem = r2
                return out

            sq_s = persist.tile([128, PO], dt.float32)
            nc.vector.tensor_scalar_mul(sq_s[:], sqx_pp[:], 0.125)
            sq0, sq1, sq2 = levels(persist, sq_s, PO, 3, "sq")
            d_vx = dram.tile([5, NS], dt.float8e4)
            for r, src in enumerate([sq0, sq0, sq1, sq1, sq2]):
                nc.sync.dma_start(
                    d_vx[r, :].rearrange("(pi po) -> pi po", pi=128), src[:]
                )
            # slabbed DoubleRow layout [4, 2, NS]; memset 4.0 covers the
            # lhsT "b" rows (5,6) and the pad row 7
            ext_l = persist.tile([4, 2, NS], dt.float8e4)
            nc.vector.memset(ext_l[:], 4.0)
            nc.sync.dma_start(ext_l[:, 0, :], d_vx[0:4, :])
            nc.sync.dma_start(ext_l[0:1, 1, :], d_vx[4:5, :])

            ry_s = persist.tile([128, BW], dt.float32)
            nc.vector.tensor_scalar_mul(ry_s[:], rsy[:], 8.0)
            ry0, ry1 = levels(persist, ry_s, BW, 2, "ry")
            b_s = persist.tile([128, BW], dt.float32)
            nc.vector.tensor_tensor(b_s[:], sqy_pp[:], rsy[:], OP.mult)
            nc.vector.tensor_scalar_mul(b_s[:], b_s[:], 0.25)
            b0, b1 = levels(persist, b_s, BW, 2, "b")
            d_vy = dram.tile([7, MS], dt.float8e4)
            for r, src in enumerate([ry0, ry1, ry0, ry1, ry0, b0, b1]):
                nc.scalar.dma_start(
                    d_vy[r, :].rearrange("(a b) -> a b", a=128), src[:]
                )
            ext_r = persist.tile([4, 2, MS], dt.float8e4)
            nc.vector.memset(ext_r[:], 0.0)
            nc.scalar.dma_start(ext_r[:, 0, :], d_vy[0:4, :])
            nc.scalar.dma_start(ext_r[0:3, 1, :], d_vy[4:7, :])

            # last y-quant tile after the tiny vec ops
            nc.vector.tensor_tensor(
                yp8[:, 3, :], yT_sb[:, 3, :], ry_bc[:], OP.mult
            )

            # ---------------- main loop ----------------
            for po in range(PO):
                acc = psum_p.tile([128, 2048], dt.float32, tag="acc",
                                  name=f"acc{po}")
                for q in range(KQ):
                    for jt in range(JT):
                        nc.tensor.matmul(
                            acc[:, jt * 512:(jt + 1) * 512],
                            lhsT=xp8[:, 2 * q:2 * q + 2, po * 128:(po + 1) * 128],
                            rhs=yp8[:, 2 * q:2 * q + 2, jt * 512:(jt + 1) * 512],
                            start=(q == 0),
                            stop=False,
                            perf_mode=DR,
                        )
                for jt in range(JT):
                    nc.tensor.matmul(
                        acc[:, jt * 512:(jt + 1) * 512],
                        lhsT=ext_l[:, :, po * 128:(po + 1) * 128],
                        rhs=ext_r[:, :, jt * 512:(jt + 1) * 512],
                        start=False,
                        stop=True,
                        perf_mode=DR,
                    )
                ot = main.tile([128, 2048], dt.float16, tag="ot")
                # split the epilogue for the last two po so the tail
                # (recip -> stt -> store) pipelines at half-po grain
                halves = 2 if po >= PO - 2 else 1
                hw_ = 2048 // halves
                for h in range(halves):
                    sl = slice(h * hw_, (h + 1) * hw_)
                    r16 = main.tile([128, hw_], dt.float16, tag=f"r16_{halves}")
                    _act_recip(nc, r16[:], acc[:, sl], rsx_s[:, po:po + 1],
                               inv_m1)
                    nc.vector.scalar_tensor_tensor(
                        ot[:, sl], r16[:], c0, r16[:], OP.add, OP.mult
                    )
                nc.sync.dma_start(out_v[:, po, :], ot[:])

    nc.compile()
    return nc


def _estimate_R(x, y, sample_x, sample_y, scale):
    rng = np.random.default_rng(12345)
    ii = rng.integers(0, x.shape[0], 4096)
    jj = rng.integers(0, y.shape[0], 4096)
    xs = np.asarray(x)[ii].astype(np.float64)
    ys = np.asarray(y)[jj].astype(np.float64)
    dd = ((xs - ys) ** 2).sum(axis=1)
    sxs = np.clip(
        np.log1p(np.exp(np.asarray(sample_x)[ii].astype(np.float64)
                        @ np.asarray(scale).reshape(-1))),
        SOFTPLUS_MIN, SOFTPLUS_MAX,
    )
    sys_ = np.clip(
        np.log1p(np.exp(np.asarray(sample_y)[jj].astype(np.float64)
                        @ np.asarray(scale).reshape(-1))),
        SOFTPLUS_MIN, SOFTPLUS_MAX,
    )
    res_s = 1.0 / (1.0 + dd / np.sqrt(sxs * sys_))
    return float(min(1.0, max(3.0 * res_s.max(), 0.01)))


def kernel(x, y, sample_x, sample_y, scale, cutoff, phi):
    from concourse.bass_utils import run_bass_kernel_spmd

    phi_val = float(np.asarray(phi).reshape(-1)[0])
    cutoff_val = float(np.clip(np.asarray(cutoff).reshape(-1)[0], 0.0, 1000.0))
    R = _estimate_R(x, y, sample_x, sample_y, scale)

    key = (phi_val, cutoff_val, round(np.log2(R), 1))
    if key not in _CACHE:
        _CACHE[key] = _build(phi_val, cutoff_val, R)
    nc = _CACHE[key]

    x = np.asarray(x, dtype=np.float32)
    y = np.asarray(y, dtype=np.float32)
    sample_x = np.ascontiguousarray(np.asarray(sample_x, dtype=np.float32))
    sample_y = np.ascontiguousarray(np.asarray(sample_y, dtype=np.float32))
    scale = np.ascontiguousarray(np.asarray(scale, dtype=np.float32)).reshape(1, S)

    bf16 = ml_dtypes.bfloat16
    xT_shards = [x[cx * NS:(cx + 1) * NS].T.astype(bf16) for cx in range(XB)]
    yT_shards = [y[cy * MS:(cy + 1) * MS].T.astype(bf16) for cy in range(YB)]
    sx_shards = [sample_x[cx * NS:(cx + 1) * NS] for cx in range(XB)]
    sy_shards = [sample_y[cy * MS:(cy + 1) * MS] for cy in range(YB)]

    in_maps = []
    for c in range(CORES):
        cx, cy = divmod(c, YB)
        in_maps.append(
            {
                "xT_shard": xT_shards[cx],
                "yT_shard": yT_shards[cy],
                "sample_x_shard": sx_shards[cx],
                "sample_y_shard": sy_shards[cy],
                "scale_full": scale,
            }
        )

    trace = bool(int(os.environ.get("KERNEL_TRACE", "0")))
    r = run_bass_kernel_spmd(nc, in_maps, core_ids=list(range(CORES)), trace=trace)
    kernel.last_results = r
    out = np.empty((N, M), dtype=np.float32)
    for c in range(CORES):
        cx, cy = divmod(c, YB)
        out[cx * NS:(cx + 1) * NS, cy * MS:(cy + 1) * MS] = (
            r.results[c]["out_shard"].astype(np.float32)
        )
    return out


if __name__ == "__main__":
    rng = np.random.default_rng(0)
    ins = {
        "x": rng.standard_normal((N, D), dtype=np.float32),
        "y": rng.standard_normal((M, D), dtype=np.float32),
        "sample_x": rng.random((N, S), dtype=np.float32),
        "sample_y": rng.random((M, S), dtype=np.float32),
        "scale": rng.random((S,), dtype=np.float32),
        "cutoff": np.full((1,), 0.1, dtype=np.float32),
        "phi": np.ones((1,), dtype=np.float32),
    }
    o = kernel(**ins)
    print(o.shape, o.dtype, o[:2, :4])
